# revision 10
# baseline (speedup 1.0000x reference)
"""Trainium2 Bass kernel for nn_DescriptorLoss (descriptor hinge loss over a
doubly-normalized correlation volume).

Decomposition (validated to ~1e-7 rel vs the jax reference):
  - Only wrap_desc needs L2-normalizing (the desc normalization cancels inside
    the first row-normalize of relu(corr)); done on host (2.4 MFLOP).
  - Sharding: image b -> core group (0-3 / 4-7); each core takes 1200 of the
    4800 query cells of its image, padded to 1280 (10 tiles of 128).
  - Phase A (device): raw = desc_q^T @ wn  (256-deep matmul, fp32),
    r2 = relu(raw)^2 via DVE max + ACT Square(accum->rn2 row norms),
    cn2[k] = sum_q r2 * inv_rn^2 via per-128-key-slice matmuls (r2 as bf16
    stationary operand, inv_rn^2 moving) accumulated in PSUM [128,38].
  - AllReduce cn2 over the 4-core image group (19KB).
  - inv_cn = 1/max(sqrt(cn2),eps); transposed on PE and broadcast to
    [128, 4864] via K=1 ones-matmuls; wn *= inv_cn in place (DVE).
  - Phase B (device): raw2 = desc_q^T @ (wn*inv_cn); one ACT pass computes
    relu(raw2*inv_rn - 0.2) in place with accum -> per-row loss partials.
  - Host: sums loss partials and adds the sparse S-correction
    sum_S [250*relu(1-d2) - relu(d2-0.2)] where S (<=4 keys/query) depends
    only on H; d2 at those pairs is recomputed on host in fp64 from the
    device's inv_rn/inv_cn stats.

Self-contained: only needs numpy + concourse (on PYTHONPATH in this env).
"""

import numpy as np

import concourse.bacc as bacc
import concourse.bass as bass
import concourse.mybir as mybir
import concourse.tile as tile
from concourse.alu_op_type import AluOpType
from concourse.bass_utils import run_bass_kernel_spmd
from concourse.masks import make_identity

# ---- problem constants (hardcoded per contract) ----
B, D, HC, WC = 2, 256, 60, 80
N = HC * WC                       # 4800 cells per image
NPAD = 4864                       # 38 * 128
QLOC = N // 4                     # 1200 queries per core
QPAD = 1280                       # 10 * 128
NQT = QPAD // 128                 # 10 query tiles
NSLICE = NPAD // 128              # 38 key slices
GROUPS = [(0, 1536), (1536, 1536), (3072, 1536), (4608, 256)]
NG = len(GROUPS)
EPS = 1e-12
LAMBDA_D, POS_MARGIN, NEG_MARGIN = 250.0, 1.0, 0.2
BLOCK, DIST_THRESH = 8, 7.5
N_CORES = 8

FP32 = mybir.dt.float32
BF16 = mybir.dt.bfloat16
AF = mybir.ActivationFunctionType

_NC_CACHE = {}


def _build_nc(mm_dt=FP32):
    nc = bacc.Bacc("TRN2", target_bir_lowering=False, debug=False,
                   num_devices=N_CORES)

    dq_d = nc.dram_tensor("dq", [2, 128, QPAD], mm_dt, kind="ExternalInput")
    wn_d = nc.dram_tensor("wnd", [2, 128, NPAD], mm_dt, kind="ExternalInput")
    invrn_d = nc.dram_tensor("invrn", [128, NQT], FP32, kind="ExternalOutput")
    invcn_d = nc.dram_tensor("invcn", [128, NSLICE], FP32, kind="ExternalOutput")
    lossacc_d = nc.dram_tensor("lossacc", [128, NQT * NG], FP32,
                               kind="ExternalOutput")

    groups_ar = [[0, 1, 2, 3], [4, 5, 6, 7]]

    with tile.TileContext(nc) as tc:
        with (
            tc.tile_pool(name="const", bufs=1) as constp,
            tc.tile_pool(name="wn", bufs=1) as wnp,
            tc.tile_pool(name="dq", bufs=1) as dqp,
            tc.tile_pool(name="r", bufs=3) as rp,
            tc.tile_pool(name="r2", bufs=2) as r2p,
            tc.tile_pool(name="small", bufs=2) as sp,
            tc.tile_pool(name="persist", bufs=1) as pp,
            tc.tile_pool(name="mmps", bufs=2, space="PSUM") as mmps,
            tc.tile_pool(name="aux", bufs=2, space="PSUM") as auxps,
            tc.tile_pool(name="dram", bufs=1, space="DRAM") as dramp,
        ):
            ident = constp.tile([128, 128], FP32)
            make_identity(nc, ident[:])
            ones1 = constp.tile([1, 128], FP32)
            nc.gpsimd.memset(ones1[:], 1.0)
            negm = constp.tile([128, 1], FP32)
            nc.gpsimd.memset(negm[:], -NEG_MARGIN)

            wn = [wnp.tile([128, NPAD], mm_dt, name=f"wn{c}") for c in range(2)]
            dq = [dqp.tile([128, QPAD], mm_dt, name=f"dq{c}") for c in range(2)]
            for c in range(2):
                nc.sync.dma_start(dq[c][:], dq_d[c])
                for (k0, kw) in GROUPS:
                    nc.sync.dma_start(wn[c][:, k0:k0 + kw], wn_d[c, :, k0:k0 + kw])

            invrn_sb = pp.tile([128, NQT], FP32)
            lossacc_sb = pp.tile([128, NQT * NG], FP32)
            cnacc_sb = pp.tile([128, NSLICE], FP32)
            nc.gpsimd.memset(cnacc_sb[:], 0.0)

            # ---------------- Phase A ----------------
            for qt in range(NQT):
                q0 = qt * 128
                r2t = r2p.tile([128, NPAD], BF16, name="r2t")
                rnacc = sp.tile([128, NG], FP32, name="rnacc")
                for g, (k0, kw) in enumerate(GROUPS):
                    ps = mmps.tile([128, 1536], FP32, name="mps", tag="mps")
                    for s in range(0, kw, 512):
                        sw = min(512, kw - s)
                        for c in range(2):
                            nc.tensor.matmul(
                                ps[:, s:s + sw],
                                lhsT=dq[c][:, q0:q0 + 128],
                                rhs=wn[c][:, k0 + s:k0 + s + sw],
                                start=(c == 0), stop=(c == 1))
                    rt = rp.tile([128, 1536], FP32, name="rt")
                    nc.vector.tensor_scalar_max(rt[:, :kw], ps[:, :kw], 0.0)
                    nc.scalar.activation(r2t[:, k0:k0 + kw], rt[:, :kw],
                                         AF.Square,
                                         accum_out=rnacc[:, g:g + 1])
                # inv_rn for this q tile
                rn2 = sp.tile([128, 1], FP32, name="rn2")
                nc.vector.tensor_reduce(rn2[:], rnacc[:],
                                        axis=mybir.AxisListType.X,
                                        op=AluOpType.add)
                rn = sp.tile([128, 1], FP32, name="rn")
                nc.scalar.activation(rn[:], rn2[:], AF.Sqrt)
                nc.vector.tensor_scalar_max(rn[:], rn[:], EPS)
                nc.vector.reciprocal(invrn_sb[:, qt:qt + 1], rn[:])
                irs = sp.tile([128, 1], FP32, name="irs")
                nc.vector.tensor_tensor(irs[:], invrn_sb[:, qt:qt + 1],
                                        invrn_sb[:, qt:qt + 1], AluOpType.mult)
                irs_bf = sp.tile([128, 1], BF16, name="irsbf")
                nc.vector.tensor_copy(irs_bf[:], irs[:])
                cn_ps = auxps.tile([128, NSLICE], FP32, name="cnps", tag="aux")
                for s in range(NSLICE):
                    nc.tensor.matmul(
                        cn_ps[:, s:s + 1],
                        lhsT=r2t[:, s * 128:(s + 1) * 128],
                        rhs=irs_bf[:],
                        start=True, stop=True)
                nc.vector.tensor_tensor(cnacc_sb[:], cnacc_sb[:], cn_ps[:],
                                        AluOpType.add)

            # ---------------- AllReduce cn2 ----------------
            cc_in = dramp.tile([128, NSLICE], FP32, name="ccin")
            cc_out = dramp.tile([128, NSLICE], FP32, name="ccout")
            nc.sync.dma_start(cc_in[:], cnacc_sb[:])
            nc.gpsimd.collective_compute(
                "AllReduce", AluOpType.add, replica_groups=groups_ar,
                ins=[cc_in.opt()], outs=[cc_out.opt()])
            cn2g = sp.tile([128, NSLICE], FP32, name="cn2g")
            nc.sync.dma_start(cn2g[:], cc_out[:])

            # inv_cn = 1/max(sqrt(cn2), eps)   [128, 38] (key k = s*128 + p)
            invcn_sb = pp.tile([128, NSLICE], FP32)
            cnr = sp.tile([128, NSLICE], FP32, name="cnr")
            nc.scalar.activation(cnr[:], cn2g[:], AF.Sqrt)
            nc.vector.tensor_scalar_max(cnr[:], cnr[:], EPS)
            nc.vector.reciprocal(invcn_sb[:], cnr[:])

            # transpose [128,38] -> [38,128], then K=1 broadcast matmuls,
            # then wn *= inv_cn in place
            t_ps = auxps.tile([NSLICE, 128], FP32, name="tps", tag="aux")
            nc.tensor.transpose(t_ps[:], invcn_sb[:], ident[:])
            t_sb = sp.tile([NSLICE, 128], FP32, name="tsb")
            nc.scalar.activation(t_sb[:], t_ps[:], AF.Copy)
            # flatten [38,128] (partition-major) to a single [1, 4864] row so
            # the K=1 broadcast matmuls read from base partition 0
            t_row = sp.tile([1, NPAD], FP32, name="trow")
            nc.sync.dma_start(t_row[:], t_sb[:])
            for g, (k0, kw) in enumerate(GROUPS):
                bps = mmps.tile([128, 1536], FP32, name="bps", tag="mps")
                for s in range(0, kw, 512):
                    sw = min(512, kw - s)
                    nc.tensor.matmul(bps[:, s:s + sw],
                                     lhsT=ones1[:],
                                     rhs=t_row[:, k0 + s:k0 + s + sw],
                                     start=True, stop=True)
                for c in range(2):
                    nc.vector.tensor_tensor(wn[c][:, k0:k0 + kw],
                                            wn[c][:, k0:k0 + kw],
                                            bps[:, :kw], AluOpType.mult)

            # ---------------- Phase B ----------------
            for qt in range(NQT):
                q0 = qt * 128
                for g, (k0, kw) in enumerate(GROUPS):
                    ps = mmps.tile([128, 1536], FP32, name="mps", tag="mps")
                    for s in range(0, kw, 512):
                        sw = min(512, kw - s)
                        for c in range(2):
                            nc.tensor.matmul(
                                ps[:, s:s + sw],
                                lhsT=dq[c][:, q0:q0 + 128],
                                rhs=wn[c][:, k0 + s:k0 + s + sw],
                                start=(c == 0), stop=(c == 1))
                    nc.scalar.activation(
                        ps[:, :kw], ps[:, :kw], AF.Relu,
                        bias=negm[:], scale=invrn_sb[:, qt:qt + 1],
                        accum_out=lossacc_sb[:, qt * NG + g:qt * NG + g + 1])

            nc.sync.dma_start(invrn_d[:], invrn_sb[:])
            nc.sync.dma_start(invcn_d[:], invcn_sb[:])
            nc.sync.dma_start(lossacc_d[:], lossacc_sb[:])

    nc.compile()
    return nc


def get_nc(mm_dt=FP32):
    key = str(mm_dt)
    if key not in _NC_CACHE:
        _NC_CACHE[key] = _build_nc(mm_dt)
    return _NC_CACHE[key]


def _host_prep(desc, wrap_desc):
    """Returns per-core input maps. Core c handles image c//4, queries
    [1200*(c%4), 1200*(c%4+1)) of that image."""
    descf = desc.reshape(B, D, N)
    wrapf = wrap_desc.reshape(B, D, N)
    wnorm = np.sqrt((wrapf.astype(np.float32) ** 2).sum(1))
    wn = (wrapf / np.maximum(wnorm, EPS)[:, None, :]).astype(np.float32)

    wn_pad = np.zeros((B, 2, 128, NPAD), np.float32)
    wn_pad[:, 0, :, :N] = wn[:, :128, :]
    wn_pad[:, 1, :, :N] = wn[:, 128:, :]

    in_maps = []
    for c in range(N_CORES):
        img, part = c // 4, c % 4
        q0 = part * QLOC
        dqc = np.zeros((2, 128, QPAD), np.float32)
        sl = descf[img][:, q0:q0 + QLOC].astype(np.float32)
        dqc[0, :, :QLOC] = sl[:128]
        dqc[1, :, :QLOC] = sl[128:]
        in_maps.append({"dq": dqc, "wnd": wn_pad[img]})
    return in_maps, wn


def _s_correction(desc, H, wn, invrn_img, invcn_img):
    """Sparse S-term computed on host (fp64 matmul over <=~16K pairs/image,
    using the device's inv_rn / inv_cn stats)."""
    descf = desc.reshape(B, D, N)
    ii, jj = np.meshgrid(np.arange(HC), np.arange(WC), indexing="ij")
    coords = (np.stack([ii, jj], -1).astype(np.float32) * BLOCK + BLOCK // 2)
    xy1 = np.concatenate([coords[..., 1:2], coords[..., 0:1],
                          np.ones((HC, WC, 1), np.float32)], -1).reshape(N, 3)
    cflat = coords.reshape(N, 2)

    corr = 0.0
    for img in range(B):
        w = (H[img].astype(np.float32) @ xy1.T.astype(np.float32)).T
        wxy = w[:, :2] / w[:, 2:3]
        warp = np.stack([wxy[:, 1], wxy[:, 0]], -1).astype(np.float32)
        diff = cflat[None, :, :] - warp[:, None, :]
        dist = np.sqrt((diff.astype(np.float32) ** 2).sum(-1))
        qs, ks = np.nonzero(dist <= DIST_THRESH)
        if len(qs) == 0:
            continue
        rawg = (descf[img][:, qs].astype(np.float64)
                * wn[img][:, ks].astype(np.float64)).sum(0)
        d2g = (np.maximum(rawg, 0.0)
               * invcn_img[img][ks].astype(np.float64)
               * invrn_img[img][qs].astype(np.float64))
        corr += (LAMBDA_D * np.maximum(POS_MARGIN - d2g, 0.0)
                 - np.maximum(d2g - NEG_MARGIN, 0.0)).sum()
    return corr


def kernel(desc, wrap_desc, H):
    desc = np.asarray(desc, np.float32)
    wrap_desc = np.asarray(wrap_desc, np.float32)
    H = np.asarray(H, np.float32)

    in_maps, wn = _host_prep(desc, wrap_desc)
    nc = get_nc()
    res = run_bass_kernel_spmd(nc, in_maps, list(range(N_CORES))).results

    dense = 0.0
    invrn_img = [np.empty(N, np.float32) for _ in range(B)]
    invcn_img = []
    for c in range(N_CORES):
        img, part = c // 4, c % 4
        dense += res[c]["lossacc"].astype(np.float64).sum()
        # invrn [128, NQT]: local query l = qt*128 + p
        loc = res[c]["invrn"].T.reshape(QPAD)[:QLOC]
        invrn_img[img][part * QLOC:(part + 1) * QLOC] = loc
    for img in range(B):
        # invcn [128, 38]: key k = s*128 + p -> transpose then flatten
        invcn_img.append(res[img * 4]["invcn"].T.reshape(NPAD)[:N])

    corr = _s_correction(desc, H, wn, invrn_img, invcn_img)
    total = (dense + corr) / float(N * N)
    return np.asarray(total, dtype=np.float32)


# revision 13
# speedup vs baseline: 1.3650x; 1.3650x over previous
"""Trainium2 Bass kernel for nn_DescriptorLoss (descriptor hinge loss over a
doubly-normalized correlation volume).

Decomposition (validated to ~1e-7 rel vs the jax reference):
  - Only wrap_desc needs L2-normalizing (the desc normalization cancels inside
    the first row-normalize of relu(corr)); done on host (2.4 MFLOP).
  - Sharding: image b -> core group (0-3 / 4-7); each core takes 1200 of the
    4800 query cells of its image, padded to 1280 (10 tiles of 128).
  - Phase A (device): raw = desc_q^T @ wn  (256-deep matmul, fp32),
    r2 = relu(raw)^2 via DVE max + ACT Square(accum->rn2 row norms),
    cn2[k] = sum_q r2 * inv_rn^2 via per-128-key-slice matmuls (r2 as bf16
    stationary operand, inv_rn^2 moving) accumulated in PSUM [128,38].
  - AllReduce cn2 over the 4-core image group (19KB).
  - inv_cn = 1/max(sqrt(cn2),eps); transposed on PE and broadcast to
    [128, 4864] via K=1 ones-matmuls; wn *= inv_cn in place (DVE).
  - Phase B (device): raw2 = desc_q^T @ (wn*inv_cn); one ACT pass computes
    relu(raw2*inv_rn - 0.2) in place with accum -> per-row loss partials.
  - Host: sums loss partials and adds the sparse S-correction
    sum_S [250*relu(1-d2) - relu(d2-0.2)] where S (<=4 keys/query) depends
    only on H; d2 at those pairs is recomputed on host in fp64 from the
    device's inv_rn/inv_cn stats.

Self-contained: only needs numpy + concourse (on PYTHONPATH in this env).
"""

import numpy as np

import concourse.bacc as bacc
import concourse.bass as bass
import concourse.mybir as mybir
import concourse.tile as tile
from concourse.alu_op_type import AluOpType
from concourse.bass_utils import run_bass_kernel_spmd
from concourse.masks import make_identity

# ---- problem constants (hardcoded per contract) ----
B, D, HC, WC = 2, 256, 60, 80
N = HC * WC                       # 4800 cells per image
NPAD = 4864                       # 38 * 128
QLOC = N // 4                     # 1200 queries per core
QPAD = 1280                       # 10 * 128
NQT = QPAD // 128                 # 10 query tiles
NSLICE = NPAD // 128              # 38 key slices
GROUPS = [(0, 1536), (1536, 1536), (3072, 1536), (4608, 256)]
NG = len(GROUPS)
EPS = 1e-12
LAMBDA_D, POS_MARGIN, NEG_MARGIN = 250.0, 1.0, 0.2
BLOCK, DIST_THRESH = 8, 7.5
N_CORES = 8

FP32 = mybir.dt.float32
BF16 = mybir.dt.bfloat16
AF = mybir.ActivationFunctionType

_NC_CACHE = {}


def _build_nc(mm_dt=FP32):
    nc = bacc.Bacc("TRN2", target_bir_lowering=False, debug=False,
                   num_devices=N_CORES)

    dq_d = nc.dram_tensor("dq", [2, 128, QPAD], mm_dt, kind="ExternalInput")
    wn_d = nc.dram_tensor("wnd", [2, 128, NPAD], mm_dt, kind="ExternalInput")
    invrn_d = nc.dram_tensor("invrn", [128, NQT], FP32, kind="ExternalOutput")
    invcn_d = nc.dram_tensor("invcn", [128, NSLICE], FP32, kind="ExternalOutput")
    lossacc_d = nc.dram_tensor("lossacc", [128, NQT * NG], FP32,
                               kind="ExternalOutput")

    groups_ar = [[0, 1, 2, 3], [4, 5, 6, 7]]

    with tile.TileContext(nc) as tc:
        with (
            tc.tile_pool(name="const", bufs=1) as constp,
            tc.tile_pool(name="wn", bufs=1) as wnp,
            tc.tile_pool(name="dq", bufs=1) as dqp,
            tc.tile_pool(name="r", bufs=3) as rp,
            tc.tile_pool(name="r2", bufs=2) as r2p,
            tc.tile_pool(name="small", bufs=2) as sp,
            tc.tile_pool(name="persist", bufs=1) as pp,
            tc.tile_pool(name="mmps", bufs=2, space="PSUM") as mmps,
            tc.tile_pool(name="aux", bufs=2, space="PSUM") as auxps,
            tc.tile_pool(name="dram", bufs=1, space="DRAM") as dramp,
        ):
            ident = constp.tile([128, 128], FP32)
            make_identity(nc, ident[:])
            ones1 = constp.tile([1, 128], FP32)
            nc.gpsimd.memset(ones1[:], 1.0)
            negm = constp.tile([128, 1], FP32)
            nc.gpsimd.memset(negm[:], -NEG_MARGIN)

            wn = [wnp.tile([128, NPAD], mm_dt, name=f"wn{c}") for c in range(2)]
            dq = [dqp.tile([128, QPAD], mm_dt, name=f"dq{c}") for c in range(2)]
            # bf16 copies for phase B (the dense relu(d2-0.2) term is ~0 for
            # normalized correlation values, so B tolerates bf16)
            wn2 = [wnp.tile([128, NPAD], BF16, name=f"wn2{c}") for c in range(2)]
            dqb = [dqp.tile([128, QPAD], BF16, name=f"dqb{c}") for c in range(2)]
            for c in range(2):
                nc.sync.dma_start(dq[c][:], dq_d[c])
                for (k0, kw) in GROUPS:
                    nc.sync.dma_start(wn[c][:, k0:k0 + kw], wn_d[c, :, k0:k0 + kw])
                nc.vector.tensor_copy(dqb[c][:], dq[c][:])

            invrn_sb = pp.tile([128, NQT], FP32)
            lossacc_sb = pp.tile([128, NQT * NG], FP32)
            cnacc_sb = pp.tile([128, NSLICE], FP32)
            nc.gpsimd.memset(cnacc_sb[:], 0.0)

            # ---------------- Phase A ----------------
            for qt in range(NQT):
                q0 = qt * 128
                r2t = r2p.tile([128, NPAD], BF16, name="r2t")
                rnacc = sp.tile([128, NG], FP32, name="rnacc")
                for g, (k0, kw) in enumerate(GROUPS):
                    ps = mmps.tile([128, 1536], FP32, name="mps", tag="mps")
                    for s in range(0, kw, 512):
                        sw = min(512, kw - s)
                        for c in range(2):
                            nc.tensor.matmul(
                                ps[:, s:s + sw],
                                lhsT=dq[c][:, q0:q0 + 128],
                                rhs=wn[c][:, k0 + s:k0 + s + sw],
                                start=(c == 0), stop=(c == 1))
                    rt = rp.tile([128, 1536], FP32, name="rt")
                    nc.vector.tensor_scalar_max(rt[:, :kw], ps[:, :kw], 0.0)
                    nc.scalar.activation(r2t[:, k0:k0 + kw], rt[:, :kw],
                                         AF.Square,
                                         accum_out=rnacc[:, g:g + 1])
                # inv_rn for this q tile
                rn2 = sp.tile([128, 1], FP32, name="rn2")
                nc.vector.tensor_reduce(rn2[:], rnacc[:],
                                        axis=mybir.AxisListType.X,
                                        op=AluOpType.add)
                rn = sp.tile([128, 1], FP32, name="rn")
                nc.scalar.activation(rn[:], rn2[:], AF.Sqrt)
                nc.vector.tensor_scalar_max(rn[:], rn[:], EPS)
                nc.vector.reciprocal(invrn_sb[:, qt:qt + 1], rn[:])
                irs = sp.tile([128, 1], FP32, name="irs")
                nc.vector.tensor_tensor(irs[:], invrn_sb[:, qt:qt + 1],
                                        invrn_sb[:, qt:qt + 1], AluOpType.mult)
                irs_bf = sp.tile([128, 1], BF16, name="irsbf")
                nc.vector.tensor_copy(irs_bf[:], irs[:])
                cn_ps = auxps.tile([128, NSLICE], FP32, name="cnps", tag="aux")
                for s in range(NSLICE):
                    nc.tensor.matmul(
                        cn_ps[:, s:s + 1],
                        lhsT=r2t[:, s * 128:(s + 1) * 128],
                        rhs=irs_bf[:],
                        start=True, stop=True)
                nc.vector.tensor_tensor(cnacc_sb[:], cnacc_sb[:], cn_ps[:],
                                        AluOpType.add)

            # ---------------- AllReduce cn2 ----------------
            cc_in = dramp.tile([128, NSLICE], FP32, name="ccin")
            cc_out = dramp.tile([128, NSLICE], FP32, name="ccout")
            nc.sync.dma_start(cc_in[:], cnacc_sb[:])
            nc.gpsimd.collective_compute(
                "AllReduce", AluOpType.add, replica_groups=groups_ar,
                ins=[cc_in.opt()], outs=[cc_out.opt()])
            cn2g = sp.tile([128, NSLICE], FP32, name="cn2g")
            nc.sync.dma_start(cn2g[:], cc_out[:])

            # inv_cn = 1/max(sqrt(cn2), eps)   [128, 38] (key k = s*128 + p)
            invcn_sb = pp.tile([128, NSLICE], FP32)
            cnr = sp.tile([128, NSLICE], FP32, name="cnr")
            nc.scalar.activation(cnr[:], cn2g[:], AF.Sqrt)
            nc.vector.tensor_scalar_max(cnr[:], cnr[:], EPS)
            nc.vector.reciprocal(invcn_sb[:], cnr[:])

            # transpose [128,38] -> [38,128], then K=1 broadcast matmuls,
            # then wn *= inv_cn in place
            t_ps = auxps.tile([NSLICE, 128], FP32, name="tps", tag="aux")
            nc.tensor.transpose(t_ps[:], invcn_sb[:], ident[:])
            t_sb = sp.tile([NSLICE, 128], FP32, name="tsb")
            nc.scalar.activation(t_sb[:], t_ps[:], AF.Copy)
            # flatten [38,128] (partition-major) to a single [1, 4864] row so
            # the K=1 broadcast matmuls read from base partition 0
            t_row = sp.tile([1, NPAD], FP32, name="trow")
            nc.sync.dma_start(t_row[:], t_sb[:])
            for g, (k0, kw) in enumerate(GROUPS):
                bps = mmps.tile([128, 1536], FP32, name="bps", tag="mps")
                for s in range(0, kw, 512):
                    sw = min(512, kw - s)
                    nc.tensor.matmul(bps[:, s:s + sw],
                                     lhsT=ones1[:],
                                     rhs=t_row[:, k0 + s:k0 + s + sw],
                                     start=True, stop=True)
                for c in range(2):
                    nc.vector.tensor_tensor(wn2[c][:, k0:k0 + kw],
                                            wn[c][:, k0:k0 + kw],
                                            bps[:, :kw], AluOpType.mult)

            # ---------------- Phase B (bf16) ----------------
            for qt in range(NQT):
                q0 = qt * 128
                for g, (k0, kw) in enumerate(GROUPS):
                    ps = mmps.tile([128, 1536], FP32, name="mps", tag="mps")
                    for s in range(0, kw, 512):
                        sw = min(512, kw - s)
                        for c in range(2):
                            nc.tensor.matmul(
                                ps[:, s:s + sw],
                                lhsT=dqb[c][:, q0:q0 + 128],
                                rhs=wn2[c][:, k0 + s:k0 + s + sw],
                                start=(c == 0), stop=(c == 1))
                    nc.scalar.activation(
                        ps[:, :kw], ps[:, :kw], AF.Relu,
                        bias=negm[:], scale=invrn_sb[:, qt:qt + 1],
                        accum_out=lossacc_sb[:, qt * NG + g:qt * NG + g + 1])

            nc.sync.dma_start(invrn_d[:], invrn_sb[:])
            nc.sync.dma_start(invcn_d[:], invcn_sb[:])
            nc.sync.dma_start(lossacc_d[:], lossacc_sb[:])

    nc.compile()
    return nc


def get_nc(mm_dt=FP32):
    key = str(mm_dt)
    if key not in _NC_CACHE:
        _NC_CACHE[key] = _build_nc(mm_dt)
    return _NC_CACHE[key]


def _host_prep(desc, wrap_desc):
    """Returns per-core input maps. Core c handles image c//4, queries
    [1200*(c%4), 1200*(c%4+1)) of that image."""
    descf = desc.reshape(B, D, N)
    wrapf = wrap_desc.reshape(B, D, N)
    wnorm = np.sqrt((wrapf.astype(np.float32) ** 2).sum(1))
    wn = (wrapf / np.maximum(wnorm, EPS)[:, None, :]).astype(np.float32)

    wn_pad = np.zeros((B, 2, 128, NPAD), np.float32)
    wn_pad[:, 0, :, :N] = wn[:, :128, :]
    wn_pad[:, 1, :, :N] = wn[:, 128:, :]

    in_maps = []
    for c in range(N_CORES):
        img, part = c // 4, c % 4
        q0 = part * QLOC
        dqc = np.zeros((2, 128, QPAD), np.float32)
        sl = descf[img][:, q0:q0 + QLOC].astype(np.float32)
        dqc[0, :, :QLOC] = sl[:128]
        dqc[1, :, :QLOC] = sl[128:]
        in_maps.append({"dq": dqc, "wnd": wn_pad[img]})
    return in_maps, wn


def _s_correction(desc, H, wn, invrn_img, invcn_img):
    """Sparse S-term computed on host (fp64 matmul over <=~16K pairs/image,
    using the device's inv_rn / inv_cn stats)."""
    descf = desc.reshape(B, D, N)
    ii, jj = np.meshgrid(np.arange(HC), np.arange(WC), indexing="ij")
    coords = (np.stack([ii, jj], -1).astype(np.float32) * BLOCK + BLOCK // 2)
    xy1 = np.concatenate([coords[..., 1:2], coords[..., 0:1],
                          np.ones((HC, WC, 1), np.float32)], -1).reshape(N, 3)
    cflat = coords.reshape(N, 2)

    corr = 0.0
    for img in range(B):
        w = (H[img].astype(np.float32) @ xy1.T.astype(np.float32)).T
        wxy = w[:, :2] / w[:, 2:3]
        warp = np.stack([wxy[:, 1], wxy[:, 0]], -1).astype(np.float32)
        diff = cflat[None, :, :] - warp[:, None, :]
        dist = np.sqrt((diff.astype(np.float32) ** 2).sum(-1))
        qs, ks = np.nonzero(dist <= DIST_THRESH)
        if len(qs) == 0:
            continue
        rawg = (descf[img][:, qs].astype(np.float64)
                * wn[img][:, ks].astype(np.float64)).sum(0)
        d2g = (np.maximum(rawg, 0.0)
               * invcn_img[img][ks].astype(np.float64)
               * invrn_img[img][qs].astype(np.float64))
        corr += (LAMBDA_D * np.maximum(POS_MARGIN - d2g, 0.0)
                 - np.maximum(d2g - NEG_MARGIN, 0.0)).sum()
    return corr


def kernel(desc, wrap_desc, H):
    desc = np.asarray(desc, np.float32)
    wrap_desc = np.asarray(wrap_desc, np.float32)
    H = np.asarray(H, np.float32)

    in_maps, wn = _host_prep(desc, wrap_desc)
    nc = get_nc()
    res = run_bass_kernel_spmd(nc, in_maps, list(range(N_CORES))).results

    dense = 0.0
    invrn_img = [np.empty(N, np.float32) for _ in range(B)]
    invcn_img = []
    for c in range(N_CORES):
        img, part = c // 4, c % 4
        dense += res[c]["lossacc"].astype(np.float64).sum()
        # invrn [128, NQT]: local query l = qt*128 + p
        loc = res[c]["invrn"].T.reshape(QPAD)[:QLOC]
        invrn_img[img][part * QLOC:(part + 1) * QLOC] = loc
    for img in range(B):
        # invcn [128, 38]: key k = s*128 + p -> transpose then flatten
        invcn_img.append(res[img * 4]["invcn"].T.reshape(NPAD)[:N])

    corr = _s_correction(desc, H, wn, invrn_img, invcn_img)
    total = (dense + corr) / float(N * N)
    return np.asarray(total, dtype=np.float32)


# revision 17
# speedup vs baseline: 2.0717x; 1.5176x over previous
"""Trainium2 Bass kernel for nn_DescriptorLoss (descriptor hinge loss over a
doubly-normalized correlation volume).

Decomposition (validated to ~1e-7 rel vs the jax reference):
  - Only wrap_desc needs L2-normalizing (the desc normalization cancels inside
    the first row-normalize of relu(corr)); done on host (2.4 MFLOP).
  - Sharding: image b -> core group (0-3 / 4-7); each core takes 1200 of the
    4800 query cells of its image, padded to 1280 (10 tiles of 128).
  - Phase A (device): raw = desc_q^T @ wn  (256-deep matmul, fp32),
    r2 = relu(raw)^2 via DVE max + ACT Square(accum->rn2 row norms),
    cn2[k] = sum_q r2 * inv_rn^2 via per-128-key-slice matmuls (r2 as bf16
    stationary operand, inv_rn^2 moving) accumulated in PSUM [128,38].
  - AllReduce cn2 over the 4-core image group (19KB).
  - inv_cn = 1/max(sqrt(cn2),eps); transposed on PE and broadcast to
    [128, 4864] via K=1 ones-matmuls; wn *= inv_cn in place (DVE).
  - Phase B (device): raw2 = desc_q^T @ (wn*inv_cn); one ACT pass computes
    relu(raw2*inv_rn - 0.2) in place with accum -> per-row loss partials.
  - Host: sums loss partials and adds the sparse S-correction
    sum_S [250*relu(1-d2) - relu(d2-0.2)] where S (<=4 keys/query) depends
    only on H; d2 at those pairs is recomputed on host in fp64 from the
    device's inv_rn/inv_cn stats.

Self-contained: only needs numpy + concourse (on PYTHONPATH in this env).
"""

import numpy as np

import concourse.bacc as bacc
import concourse.bass as bass
import concourse.mybir as mybir
import concourse.tile as tile
from concourse.alu_op_type import AluOpType
from concourse.bass_utils import run_bass_kernel_spmd
from concourse.masks import make_identity

# ---- problem constants (hardcoded per contract) ----
B, D, HC, WC = 2, 256, 60, 80
N = HC * WC                       # 4800 cells per image
NPAD = 4864                       # 38 * 128
QLOC = N // 4                     # 1200 queries per core
QPAD = 1280                       # 10 * 128
NQT = QPAD // 128                 # 10 query tiles
NSLICE = NPAD // 128              # 38 key slices
GROUPS = [(0, 1536), (1536, 1536), (3072, 1536), (4608, 256)]
NG = len(GROUPS)
EPS = 1e-12
LAMBDA_D, POS_MARGIN, NEG_MARGIN = 250.0, 1.0, 0.2
BLOCK, DIST_THRESH = 8, 7.5
N_CORES = 8

FP32 = mybir.dt.float32
F32R = mybir.dt.float32r
BF16 = mybir.dt.bfloat16
AF = mybir.ActivationFunctionType
A_MM_DT = F32R   # phase-A matmul operand dtype (bits are fp32 either way)

_NC_CACHE = {}


def _build_nc(mm_dt=FP32):
    nc = bacc.Bacc("TRN2", target_bir_lowering=False, debug=False,
                   num_devices=N_CORES)

    dq_d = nc.dram_tensor("dq", [2, 128, QPAD], mm_dt, kind="ExternalInput")
    wn_d = nc.dram_tensor("wnd", [2, 128, NPAD], mm_dt, kind="ExternalInput")
    invrn_d = nc.dram_tensor("invrn", [128, NQT], FP32, kind="ExternalOutput")
    invcn_d = nc.dram_tensor("invcn", [128, NSLICE], FP32, kind="ExternalOutput")
    lossacc_d = nc.dram_tensor("lossacc", [128, NQT * NG], FP32,
                               kind="ExternalOutput")

    groups_ar = [[0, 1, 2, 3], [4, 5, 6, 7]]

    with tile.TileContext(nc) as tc:
        with (
            tc.tile_pool(name="const", bufs=1) as constp,
            tc.tile_pool(name="wn", bufs=1) as wnp,
            tc.tile_pool(name="dq", bufs=1) as dqp,
            tc.tile_pool(name="r", bufs=3) as rp,
            tc.tile_pool(name="r2", bufs=2) as r2p,
            tc.tile_pool(name="small", bufs=2) as sp,
            tc.tile_pool(name="persist", bufs=1) as pp,
            tc.tile_pool(name="mmps", bufs=2, space="PSUM") as mmps,
            tc.tile_pool(name="aux", bufs=2, space="PSUM") as auxps,
            tc.tile_pool(name="dram", bufs=1, space="DRAM") as dramp,
        ):
            ident = constp.tile([128, 128], FP32)
            make_identity(nc, ident[:])
            ones1 = constp.tile([1, 128], FP32)
            nc.gpsimd.memset(ones1[:], 1.0)
            negm = constp.tile([128, 1], FP32)
            nc.gpsimd.memset(negm[:], -NEG_MARGIN)

            wn = [wnp.tile([128, NPAD], mm_dt, name=f"wn{c}") for c in range(2)]
            dq = [dqp.tile([128, QPAD], mm_dt, name=f"dq{c}") for c in range(2)]
            # bf16 copies for phase B (the dense relu(d2-0.2) term is ~0 for
            # normalized correlation values, so B tolerates bf16)
            wn2 = [wnp.tile([128, NPAD], BF16, name=f"wn2{c}") for c in range(2)]
            dqb = [dqp.tile([128, QPAD], BF16, name=f"dqb{c}") for c in range(2)]
            wnr = [wnp.tile([128, NPAD], A_MM_DT, name=f"wnr{c}") for c in range(2)]
            dqr = [dqp.tile([128, QPAD], A_MM_DT, name=f"dqr{c}") for c in range(2)]
            for c in range(2):
                nc.sync.dma_start(dq[c][:], dq_d[c])
                for (k0, kw) in GROUPS:
                    nc.sync.dma_start(wn[c][:, k0:k0 + kw], wn_d[c, :, k0:k0 + kw])
                    nc.vector.tensor_copy(wnr[c][:, k0:k0 + kw],
                                          wn[c][:, k0:k0 + kw])
                nc.vector.tensor_copy(dqb[c][:], dq[c][:])
                nc.vector.tensor_copy(dqr[c][:], dq[c][:])

            invrn_sb = pp.tile([128, NQT], FP32)
            lossacc_sb = pp.tile([128, NQT * NG], FP32)
            cnacc_sb = pp.tile([128, NSLICE], FP32)
            nc.gpsimd.memset(cnacc_sb[:], 0.0)

            # ---------------- Phase A ----------------
            for qt in range(NQT):
                q0 = qt * 128
                r2t = r2p.tile([128, NPAD], BF16, name="r2t")
                rnacc = sp.tile([128, NG], FP32, name="rnacc")
                for g, (k0, kw) in enumerate(GROUPS):
                    ps = mmps.tile([128, 1536], FP32, name="mps", tag="mps")
                    for s in range(0, kw, 512):
                        sw = min(512, kw - s)
                        for c in range(2):
                            nc.tensor.matmul(
                                ps[:, s:s + sw],
                                lhsT=dqr[c][:, q0:q0 + 128],
                                rhs=wnr[c][:, k0 + s:k0 + s + sw],
                                start=(c == 0), stop=(c == 1))
                    rt = rp.tile([128, 1536], FP32, name="rt")
                    nc.vector.tensor_scalar_max(rt[:, :kw], ps[:, :kw], 0.0)
                    nc.scalar.activation(r2t[:, k0:k0 + kw], rt[:, :kw],
                                         AF.Square,
                                         accum_out=rnacc[:, g:g + 1])
                # inv_rn for this q tile
                rn2 = sp.tile([128, 1], FP32, name="rn2")
                nc.vector.tensor_reduce(rn2[:], rnacc[:],
                                        axis=mybir.AxisListType.X,
                                        op=AluOpType.add)
                rn = sp.tile([128, 1], FP32, name="rn")
                nc.scalar.activation(rn[:], rn2[:], AF.Sqrt)
                nc.vector.tensor_scalar_max(rn[:], rn[:], EPS)
                nc.vector.reciprocal(invrn_sb[:, qt:qt + 1], rn[:])
                irs = sp.tile([128, 1], FP32, name="irs")
                nc.vector.tensor_tensor(irs[:], invrn_sb[:, qt:qt + 1],
                                        invrn_sb[:, qt:qt + 1], AluOpType.mult)
                irs_bf = sp.tile([128, 1], BF16, name="irsbf")
                nc.vector.tensor_copy(irs_bf[:], irs[:])
                cn_ps = auxps.tile([128, NSLICE], FP32, name="cnps", tag="aux")
                for s in range(NSLICE):
                    nc.tensor.matmul(
                        cn_ps[:, s:s + 1],
                        lhsT=r2t[:, s * 128:(s + 1) * 128],
                        rhs=irs_bf[:],
                        start=True, stop=True)
                nc.vector.tensor_tensor(cnacc_sb[:], cnacc_sb[:], cn_ps[:],
                                        AluOpType.add)

            # ---------------- AllReduce cn2 ----------------
            cc_in = dramp.tile([128, NSLICE], FP32, name="ccin")
            cc_out = dramp.tile([128, NSLICE], FP32, name="ccout")
            nc.sync.dma_start(cc_in[:], cnacc_sb[:])
            nc.gpsimd.collective_compute(
                "AllReduce", AluOpType.add, replica_groups=groups_ar,
                ins=[cc_in.opt()], outs=[cc_out.opt()])
            cn2g = sp.tile([128, NSLICE], FP32, name="cn2g")
            nc.sync.dma_start(cn2g[:], cc_out[:])

            # inv_cn = 1/max(sqrt(cn2), eps)   [128, 38] (key k = s*128 + p)
            invcn_sb = pp.tile([128, NSLICE], FP32)
            cnr = sp.tile([128, NSLICE], FP32, name="cnr")
            nc.scalar.activation(cnr[:], cn2g[:], AF.Sqrt)
            nc.vector.tensor_scalar_max(cnr[:], cnr[:], EPS)
            nc.vector.reciprocal(invcn_sb[:], cnr[:])

            # transpose [128,38] -> [38,128], then K=1 broadcast matmuls,
            # then wn *= inv_cn in place
            t_ps = auxps.tile([NSLICE, 128], FP32, name="tps", tag="aux")
            nc.tensor.transpose(t_ps[:], invcn_sb[:], ident[:])
            t_sb = sp.tile([NSLICE, 128], FP32, name="tsb")
            nc.scalar.activation(t_sb[:], t_ps[:], AF.Copy)
            # flatten [38,128] (partition-major) to a single [1, 4864] row so
            # the K=1 broadcast matmuls read from base partition 0
            t_row = sp.tile([1, NPAD], FP32, name="trow")
            nc.sync.dma_start(t_row[:], t_sb[:])
            for g, (k0, kw) in enumerate(GROUPS):
                bps = mmps.tile([128, 1536], FP32, name="bps", tag="mps")
                for s in range(0, kw, 512):
                    sw = min(512, kw - s)
                    nc.tensor.matmul(bps[:, s:s + sw],
                                     lhsT=ones1[:],
                                     rhs=t_row[:, k0 + s:k0 + s + sw],
                                     start=True, stop=True)
                for c in range(2):
                    nc.vector.tensor_tensor(wn2[c][:, k0:k0 + kw],
                                            wn[c][:, k0:k0 + kw],
                                            bps[:, :kw], AluOpType.mult)

            # ---------------- Phase B (bf16) ----------------
            for qt in range(NQT):
                q0 = qt * 128
                for g, (k0, kw) in enumerate(GROUPS):
                    ps = mmps.tile([128, 1536], FP32, name="mps", tag="mps")
                    for s in range(0, kw, 512):
                        sw = min(512, kw - s)
                        for c in range(2):
                            nc.tensor.matmul(
                                ps[:, s:s + sw],
                                lhsT=dqb[c][:, q0:q0 + 128],
                                rhs=wn2[c][:, k0 + s:k0 + s + sw],
                                start=(c == 0), stop=(c == 1))
                    nc.scalar.activation(
                        ps[:, :kw], ps[:, :kw], AF.Relu,
                        bias=negm[:], scale=invrn_sb[:, qt:qt + 1],
                        accum_out=lossacc_sb[:, qt * NG + g:qt * NG + g + 1])

            nc.sync.dma_start(invrn_d[:], invrn_sb[:])
            nc.sync.dma_start(invcn_d[:], invcn_sb[:])
            nc.sync.dma_start(lossacc_d[:], lossacc_sb[:])

    nc.compile()
    return nc


def get_nc(mm_dt=FP32):
    key = str(mm_dt)
    if key not in _NC_CACHE:
        _NC_CACHE[key] = _build_nc(mm_dt)
    return _NC_CACHE[key]


def _host_prep(desc, wrap_desc):
    """Returns per-core input maps. Core c handles image c//4, queries
    [1200*(c%4), 1200*(c%4+1)) of that image."""
    descf = desc.reshape(B, D, N)
    wrapf = wrap_desc.reshape(B, D, N)
    wnorm = np.sqrt((wrapf.astype(np.float32) ** 2).sum(1))
    wn = (wrapf / np.maximum(wnorm, EPS)[:, None, :]).astype(np.float32)

    wn_pad = np.zeros((B, 2, 128, NPAD), np.float32)
    wn_pad[:, 0, :, :N] = wn[:, :128, :]
    wn_pad[:, 1, :, :N] = wn[:, 128:, :]

    in_maps = []
    for c in range(N_CORES):
        img, part = c // 4, c % 4
        q0 = part * QLOC
        dqc = np.zeros((2, 128, QPAD), np.float32)
        sl = descf[img][:, q0:q0 + QLOC].astype(np.float32)
        dqc[0, :, :QLOC] = sl[:128]
        dqc[1, :, :QLOC] = sl[128:]
        in_maps.append({"dq": dqc, "wnd": wn_pad[img]})
    return in_maps, wn


def _s_correction(desc, H, wn, invrn_img, invcn_img):
    """Sparse S-term computed on host (fp64 matmul over <=~16K pairs/image,
    using the device's inv_rn / inv_cn stats)."""
    descf = desc.reshape(B, D, N)
    ii, jj = np.meshgrid(np.arange(HC), np.arange(WC), indexing="ij")
    coords = (np.stack([ii, jj], -1).astype(np.float32) * BLOCK + BLOCK // 2)
    xy1 = np.concatenate([coords[..., 1:2], coords[..., 0:1],
                          np.ones((HC, WC, 1), np.float32)], -1).reshape(N, 3)
    cflat = coords.reshape(N, 2)

    corr = 0.0
    for img in range(B):
        w = (H[img].astype(np.float32) @ xy1.T.astype(np.float32)).T
        wxy = w[:, :2] / w[:, 2:3]
        warp = np.stack([wxy[:, 1], wxy[:, 0]], -1).astype(np.float32)
        diff = cflat[None, :, :] - warp[:, None, :]
        dist = np.sqrt((diff.astype(np.float32) ** 2).sum(-1))
        qs, ks = np.nonzero(dist <= DIST_THRESH)
        if len(qs) == 0:
            continue
        rawg = (descf[img][:, qs].astype(np.float64)
                * wn[img][:, ks].astype(np.float64)).sum(0)
        d2g = (np.maximum(rawg, 0.0)
               * invcn_img[img][ks].astype(np.float64)
               * invrn_img[img][qs].astype(np.float64))
        corr += (LAMBDA_D * np.maximum(POS_MARGIN - d2g, 0.0)
                 - np.maximum(d2g - NEG_MARGIN, 0.0)).sum()
    return corr


def kernel(desc, wrap_desc, H):
    desc = np.asarray(desc, np.float32)
    wrap_desc = np.asarray(wrap_desc, np.float32)
    H = np.asarray(H, np.float32)

    in_maps, wn = _host_prep(desc, wrap_desc)
    nc = get_nc()
    res = run_bass_kernel_spmd(nc, in_maps, list(range(N_CORES))).results

    dense = 0.0
    invrn_img = [np.empty(N, np.float32) for _ in range(B)]
    invcn_img = []
    for c in range(N_CORES):
        img, part = c // 4, c % 4
        dense += res[c]["lossacc"].astype(np.float64).sum()
        # invrn [128, NQT]: local query l = qt*128 + p
        loc = res[c]["invrn"].T.reshape(QPAD)[:QLOC]
        invrn_img[img][part * QLOC:(part + 1) * QLOC] = loc
    for img in range(B):
        # invcn [128, 38]: key k = s*128 + p -> transpose then flatten
        invcn_img.append(res[img * 4]["invcn"].T.reshape(NPAD)[:N])

    corr = _s_correction(desc, H, wn, invrn_img, invcn_img)
    total = (dense + corr) / float(N * N)
    return np.asarray(total, dtype=np.float32)


# revision 24
# speedup vs baseline: 2.1364x; 1.0312x over previous
"""Trainium2 Bass kernel for nn_DescriptorLoss (descriptor hinge loss over a
doubly-normalized correlation volume).

Decomposition (validated to ~1e-7 rel vs the jax reference):
  - Only wrap_desc needs L2-normalizing (the desc normalization cancels inside
    the first row-normalize of relu(corr)); done on host (2.4 MFLOP).
  - Sharding: image b -> core group (0-3 / 4-7); each core takes 1200 of the
    4800 query cells of its image, padded to 1280 (10 tiles of 128).
  - Phase A (device): raw = desc_q^T @ wn  (256-deep matmul, fp32),
    r2 = relu(raw)^2 via DVE max + ACT Square(accum->rn2 row norms),
    cn2[k] = sum_q r2 * inv_rn^2 via per-128-key-slice matmuls (r2 as bf16
    stationary operand, inv_rn^2 moving) accumulated in PSUM [128,38].
  - AllReduce cn2 over the 4-core image group (19KB).
  - inv_cn = 1/max(sqrt(cn2),eps); transposed on PE and broadcast to
    [128, 4864] via K=1 ones-matmuls; wn *= inv_cn in place (DVE).
  - Phase B (device): raw2 = desc_q^T @ (wn*inv_cn); one ACT pass computes
    relu(raw2*inv_rn - 0.2) in place with accum -> per-row loss partials.
  - Host: sums loss partials and adds the sparse S-correction
    sum_S [250*relu(1-d2) - relu(d2-0.2)] where S (<=4 keys/query) depends
    only on H; d2 at those pairs is recomputed on host in fp64 from the
    device's inv_rn/inv_cn stats.

Self-contained: only needs numpy + concourse (on PYTHONPATH in this env).
"""

import numpy as np

import concourse.bacc as bacc
import concourse.bass as bass
import concourse.mybir as mybir
import concourse.tile as tile
from concourse.alu_op_type import AluOpType
from concourse.bass_utils import run_bass_kernel_spmd
from concourse.masks import make_identity

# ---- problem constants (hardcoded per contract) ----
B, D, HC, WC = 2, 256, 60, 80
N = HC * WC                       # 4800 cells per image
NPAD = 4864                       # 38 * 128
QLOC = N // 4                     # 1200 queries per core
QPAD = 1280                       # 10 * 128
NQT = QPAD // 128                 # 10 query tiles
NSLICE = NPAD // 128              # 38 key slices
GROUPS = [(0, 1536), (1536, 1536), (3072, 1536), (4608, 256)]
NG = len(GROUPS)
EPS = 1e-12
LAMBDA_D, POS_MARGIN, NEG_MARGIN = 250.0, 1.0, 0.2
BLOCK, DIST_THRESH = 8, 7.5
N_CORES = 8

FP32 = mybir.dt.float32
F32R = mybir.dt.float32r
BF16 = mybir.dt.bfloat16
AF = mybir.ActivationFunctionType
A_MM_DT = F32R   # phase-A matmul operand dtype (bits are fp32 either way)

_NC_CACHE = {}


def _build_nc(mm_dt=FP32):
    nc = bacc.Bacc("TRN2", target_bir_lowering=False, debug=False,
                   num_devices=N_CORES)

    dq_d = nc.dram_tensor("dq", [2, 128, QPAD], mm_dt, kind="ExternalInput")
    wn_d = nc.dram_tensor("wnd", [2, 128, NPAD], mm_dt, kind="ExternalInput")
    invrn_d = nc.dram_tensor("invrn", [128, NQT], FP32, kind="ExternalOutput")
    invcn_d = nc.dram_tensor("invcn", [128, NSLICE], FP32, kind="ExternalOutput")
    lossacc_d = nc.dram_tensor("lossacc", [128, NQT * NG], FP32,
                               kind="ExternalOutput")

    groups_ar = [[0, 1, 2, 3], [4, 5, 6, 7]]

    with tile.TileContext(nc) as tc:
        with (
            tc.tile_pool(name="const", bufs=1) as constp,
            tc.tile_pool(name="wn", bufs=1) as wnp,
            tc.tile_pool(name="dq", bufs=1) as dqp,
            tc.tile_pool(name="r", bufs=3) as rp,
            tc.tile_pool(name="r2", bufs=2) as r2p,
            tc.tile_pool(name="small", bufs=2) as sp,
            tc.tile_pool(name="persist", bufs=1) as pp,
            tc.tile_pool(name="mmps", bufs=2, space="PSUM") as mmps,
            tc.tile_pool(name="aux", bufs=2, space="PSUM") as auxps,
            tc.tile_pool(name="dram", bufs=1, space="DRAM") as dramp,
        ):
            ident = constp.tile([128, 128], FP32)
            make_identity(nc, ident[:])
            ones1 = constp.tile([1, 128], FP32)
            nc.gpsimd.memset(ones1[:], 1.0)
            negm = constp.tile([128, 1], FP32)
            nc.gpsimd.memset(negm[:], -NEG_MARGIN)

            wn = [wnp.tile([128, NPAD], mm_dt, name=f"wn{c}") for c in range(2)]
            dq = [dqp.tile([128, QPAD], mm_dt, name=f"dq{c}") for c in range(2)]
            wnr = [wnp.tile([128, NPAD], A_MM_DT, name=f"wnr{c}") for c in range(2)]
            dqr = [dqp.tile([128, QPAD], A_MM_DT, name=f"dqr{c}") for c in range(2)]
            # bf16 copies for phase B (the dense relu(d2-0.2) term is ~0 for
            # normalized correlation values, so B tolerates bf16)
            wn2 = [wnp.tile([128, NPAD], BF16, name=f"wn2{c}") for c in range(2)]
            dqb = [dqp.tile([128, QPAD], BF16, name=f"dqb{c}") for c in range(2)]
            for c in range(2):
                nc.sync.dma_start(dq[c][:], dq_d[c])
                for (k0, kw) in GROUPS:
                    nc.sync.dma_start(wn[c][:, k0:k0 + kw], wn_d[c, :, k0:k0 + kw])
                    nc.vector.tensor_copy(wnr[c][:, k0:k0 + kw],
                                          wn[c][:, k0:k0 + kw])
                nc.vector.tensor_copy(dqb[c][:], dq[c][:])
                nc.vector.tensor_copy(dqr[c][:], dq[c][:])

            invrn_sb = pp.tile([128, NQT], FP32)
            lossacc_sb = pp.tile([128, NQT * NG], FP32)
            cnacc_sb = pp.tile([128, NSLICE], FP32)
            nc.gpsimd.memset(cnacc_sb[:], 0.0)

            # ---------------- Phase A ----------------
            for qt in range(NQT):
                q0 = qt * 128
                r2t = r2p.tile([128, NPAD], BF16, name="r2t")
                rnacc = sp.tile([128, NG], FP32, name="rnacc")
                for g, (k0, kw) in enumerate(GROUPS):
                    ps = mmps.tile([128, 1536], FP32, name="mps", tag="mps")
                    for s in range(0, kw, 512):
                        sw = min(512, kw - s)
                        for c in range(2):
                            nc.tensor.matmul(
                                ps[:, s:s + sw],
                                lhsT=dqr[c][:, q0:q0 + 128],
                                rhs=wnr[c][:, k0 + s:k0 + s + sw],
                                start=(c == 0), stop=(c == 1))
                    rt = rp.tile([128, 1536], FP32, name="rt")
                    nc.vector.tensor_scalar_max(rt[:, :kw], ps[:, :kw], 0.0)
                    nc.scalar.activation(r2t[:, k0:k0 + kw], rt[:, :kw],
                                         AF.Square,
                                         accum_out=rnacc[:, g:g + 1])
                # inv_rn for this q tile
                rn2 = sp.tile([128, 1], FP32, name="rn2")
                nc.vector.tensor_reduce(rn2[:], rnacc[:],
                                        axis=mybir.AxisListType.X,
                                        op=AluOpType.add)
                rn = sp.tile([128, 1], FP32, name="rn")
                nc.scalar.activation(rn[:], rn2[:], AF.Sqrt)
                nc.vector.tensor_scalar_max(rn[:], rn[:], EPS)
                nc.vector.reciprocal(invrn_sb[:, qt:qt + 1], rn[:])
                irs = sp.tile([128, 1], FP32, name="irs")
                nc.vector.tensor_tensor(irs[:], invrn_sb[:, qt:qt + 1],
                                        invrn_sb[:, qt:qt + 1], AluOpType.mult)
                irs_bf = sp.tile([128, 1], BF16, name="irsbf")
                nc.vector.tensor_copy(irs_bf[:], irs[:])
                cn_ps = auxps.tile([128, NSLICE], FP32, name="cnps", tag="aux")
                for s in range(NSLICE):
                    nc.tensor.matmul(
                        cn_ps[:, s:s + 1],
                        lhsT=r2t[:, s * 128:(s + 1) * 128],
                        rhs=irs_bf[:],
                        start=True, stop=True)
                nc.vector.tensor_tensor(cnacc_sb[:], cnacc_sb[:], cn_ps[:],
                                        AluOpType.add)

            # ---------------- AllReduce cn2 ----------------
            cc_in = dramp.tile([128, NSLICE], FP32, name="ccin")
            cc_out = dramp.tile([128, NSLICE], FP32, name="ccout")
            nc.sync.dma_start(cc_in[:], cnacc_sb[:])
            nc.gpsimd.collective_compute(
                "AllReduce", AluOpType.add, replica_groups=groups_ar,
                ins=[cc_in.opt()], outs=[cc_out.opt()])
            cn2g = sp.tile([128, NSLICE], FP32, name="cn2g")
            nc.sync.dma_start(cn2g[:], cc_out[:])

            # inv_cn = 1/max(sqrt(cn2), eps)   [128, 38] (key k = s*128 + p)
            invcn_sb = pp.tile([128, NSLICE], FP32)
            cnr = sp.tile([128, NSLICE], FP32, name="cnr")
            nc.scalar.activation(cnr[:], cn2g[:], AF.Sqrt)
            nc.vector.tensor_scalar_max(cnr[:], cnr[:], EPS)
            nc.vector.reciprocal(invcn_sb[:], cnr[:])

            # transpose [128,38] -> [38,128], then K=1 broadcast matmuls,
            # then wn *= inv_cn in place
            t_ps = auxps.tile([NSLICE, 128], FP32, name="tps", tag="aux")
            nc.tensor.transpose(t_ps[:], invcn_sb[:], ident[:])
            t_sb = sp.tile([NSLICE, 128], FP32, name="tsb")
            nc.scalar.activation(t_sb[:], t_ps[:], AF.Copy)
            # flatten [38,128] (partition-major) to a single [1, 4864] row so
            # the K=1 broadcast matmuls read from base partition 0
            t_row = sp.tile([1, NPAD], FP32, name="trow")
            nc.sync.dma_start(t_row[:], t_sb[:])
            for g, (k0, kw) in enumerate(GROUPS):
                bps = mmps.tile([128, 1536], FP32, name="bps", tag="mps")
                for s in range(0, kw, 512):
                    sw = min(512, kw - s)
                    nc.tensor.matmul(bps[:, s:s + sw],
                                     lhsT=ones1[:],
                                     rhs=t_row[:, k0 + s:k0 + s + sw],
                                     start=True, stop=True)
                for c in range(2):
                    nc.vector.tensor_tensor(wn2[c][:, k0:k0 + kw],
                                            wn[c][:, k0:k0 + kw],
                                            bps[:, :kw], AluOpType.mult)

            # ---------------- Phase B (bf16) ----------------
            for qt in range(NQT):
                q0 = qt * 128
                for g, (k0, kw) in enumerate(GROUPS):
                    ps = mmps.tile([128, 1536], FP32, name="mps", tag="mps")
                    for s in range(0, kw, 512):
                        sw = min(512, kw - s)
                        for c in range(2):
                            nc.tensor.matmul(
                                ps[:, s:s + sw],
                                lhsT=dqb[c][:, q0:q0 + 128],
                                rhs=wn2[c][:, k0 + s:k0 + s + sw],
                                start=(c == 0), stop=(c == 1))
                    nc.scalar.activation(
                        ps[:, :kw], ps[:, :kw], AF.Relu,
                        bias=negm[:], scale=invrn_sb[:, qt:qt + 1],
                        accum_out=lossacc_sb[:, qt * NG + g:qt * NG + g + 1])

            nc.sync.dma_start(invrn_d[:], invrn_sb[:])
            nc.sync.dma_start(invcn_d[:], invcn_sb[:])
            nc.sync.dma_start(lossacc_d[:], lossacc_sb[:])

    nc.compile()
    return nc


def get_nc(mm_dt=FP32):
    key = str(mm_dt)
    if key not in _NC_CACHE:
        _NC_CACHE[key] = _build_nc(mm_dt)
    return _NC_CACHE[key]


def _host_prep(desc, wrap_desc):
    """Returns per-core input maps. Core c handles image c//4, queries
    [1200*(c%4), 1200*(c%4+1)) of that image."""
    descf = desc.reshape(B, D, N)
    wrapf = wrap_desc.reshape(B, D, N)
    wnorm = np.sqrt((wrapf.astype(np.float32) ** 2).sum(1))
    wn = (wrapf / np.maximum(wnorm, EPS)[:, None, :]).astype(np.float32)

    wn_pad = np.zeros((B, 2, 128, NPAD), np.float32)
    wn_pad[:, 0, :, :N] = wn[:, :128, :]
    wn_pad[:, 1, :, :N] = wn[:, 128:, :]

    in_maps = []
    for c in range(N_CORES):
        img, part = c // 4, c % 4
        q0 = part * QLOC
        dqc = np.zeros((2, 128, QPAD), np.float32)
        sl = descf[img][:, q0:q0 + QLOC].astype(np.float32)
        dqc[0, :, :QLOC] = sl[:128]
        dqc[1, :, :QLOC] = sl[128:]
        in_maps.append({"dq": dqc, "wnd": wn_pad[img]})
    return in_maps, wn


def _s_correction(desc, H, wn, invrn_img, invcn_img):
    """Sparse S-term computed on host (fp64 matmul over <=~16K pairs/image,
    using the device's inv_rn / inv_cn stats)."""
    descf = desc.reshape(B, D, N)
    ii, jj = np.meshgrid(np.arange(HC), np.arange(WC), indexing="ij")
    coords = (np.stack([ii, jj], -1).astype(np.float32) * BLOCK + BLOCK // 2)
    xy1 = np.concatenate([coords[..., 1:2], coords[..., 0:1],
                          np.ones((HC, WC, 1), np.float32)], -1).reshape(N, 3)
    cflat = coords.reshape(N, 2)

    corr = 0.0
    for img in range(B):
        w = (H[img].astype(np.float32) @ xy1.T.astype(np.float32)).T
        wxy = w[:, :2] / w[:, 2:3]
        warp = np.stack([wxy[:, 1], wxy[:, 0]], -1).astype(np.float32)
        diff = cflat[None, :, :] - warp[:, None, :]
        dist = np.sqrt((diff.astype(np.float32) ** 2).sum(-1))
        qs, ks = np.nonzero(dist <= DIST_THRESH)
        if len(qs) == 0:
            continue
        rawg = (descf[img][:, qs].astype(np.float64)
                * wn[img][:, ks].astype(np.float64)).sum(0)
        d2g = (np.maximum(rawg, 0.0)
               * invcn_img[img][ks].astype(np.float64)
               * invrn_img[img][qs].astype(np.float64))
        corr += (LAMBDA_D * np.maximum(POS_MARGIN - d2g, 0.0)
                 - np.maximum(d2g - NEG_MARGIN, 0.0)).sum()
    return corr


def kernel(desc, wrap_desc, H):
    desc = np.asarray(desc, np.float32)
    wrap_desc = np.asarray(wrap_desc, np.float32)
    H = np.asarray(H, np.float32)

    in_maps, wn = _host_prep(desc, wrap_desc)
    nc = get_nc()
    res = run_bass_kernel_spmd(nc, in_maps, list(range(N_CORES))).results

    dense = 0.0
    invrn_img = [np.empty(N, np.float32) for _ in range(B)]
    invcn_img = []
    for c in range(N_CORES):
        img, part = c // 4, c % 4
        dense += res[c]["lossacc"].astype(np.float64).sum()
        # invrn [128, NQT]: local query l = qt*128 + p
        loc = res[c]["invrn"].T.reshape(QPAD)[:QLOC]
        invrn_img[img][part * QLOC:(part + 1) * QLOC] = loc
    for img in range(B):
        # invcn [128, 38]: key k = s*128 + p -> transpose then flatten
        invcn_img.append(res[img * 4]["invcn"].T.reshape(NPAD)[:N])

    corr = _s_correction(desc, H, wn, invrn_img, invcn_img)
    total = (dense + corr) / float(N * N)
    return np.asarray(total, dtype=np.float32)
